# revision 15
# baseline (speedup 1.0000x reference)
"""Trainium2 Bass kernel for nn_CliffordMeanField3DLayerExact.

Math: with C the Cl(3,0) Cayley tensor,
  mean_b   = mean_n x[b]                                  [D]
  mf_b     = Wm @ mean_b + bm                             [8]
  M_b      = einsum('ijk,j->ik', C, mf_b)                 [8,8]
  y        = x + S*( (x@Wp.T + bp) @ M_b @ Wo.T + bo )
  out      = LayerNorm(y) * gamma + beta

Rank-8 structure: y = x + (x @ Wp.T) @ Bmat_b + cc_b with
  Bmat_b = S * (M_b @ Wo.T)   [8, D]
  cc_b   = bp @ Bmat_b + S*bo [D]

Sharding: data-parallel over B; core b owns batch b (B == n_cores == 8).
Two launches: (A) per-batch column sums of x (VectorE free-dim reduces +
one ones-matmul partition reduce) -> host computes the tiny 8x8 algebra
-> (B) main kernel, per super-tile of 2x128 rows:
  xT = transpose(x_tile)  (TensorE via identity, PSUM -> bf16 SBUF copy)
  pT[8,256] = Wp @ xT     (4 accumulating bf16 matmuls)
  y_psum = ones.T@cc + pT.T @ Bmat   (2 matmuls into PSUM per sub-tile)
  y = x_tile + y_psum     (DVE scalar_tensor_tensor)
  LN stats via DVE bn_stats/bn_aggr; sqrt on ScalarE; normalize on DVE
  tensor_scalar with per-partition rstd / -mu*rstd scalars.
"""

import numpy as np
import ml_dtypes

import concourse.bass as bass
import concourse.tile as tile
from concourse import mybir
from concourse.bass_utils import run_bass_kernel_spmd

# ---------------------------------------------------------------- constants
B, N, D, MV = 8, 16384, 512, 8
SCALE = 0.1
EPS = 1e-5
P = 128                      # partitions
NT = N // P                  # 128 row-tiles per batch
F32 = mybir.dt.float32
BF16 = mybir.dt.bfloat16
BF16_NP = ml_dtypes.bfloat16

_CORES = list(range(8))


# --------------------------------------------------- multi-wait workaround
def _split_multi_waits(nc: bass.Bass):
    """This container's walrus accepts only ONE sem-wait per instruction
    (EventSemaphore: two), but Tile attaches several (e.g. WAR + queue-slot
    waits on DMAs, per-proc waits on the tail drain). Hoist extra waits
    onto single-wait NoOps inserted right before the offender on the same
    engine — the engine executes in order, so the gating is equivalent."""
    for f in nc.m.functions:
        for bb in f.blocks:
            out, changed = [], False
            for inst in bb.instructions:
                si = inst.sync_info
                cap = 2 if isinstance(inst, mybir.InstEventSemaphore) else 1
                if si is not None and len(si.on_wait) > cap:
                    waits = list(si.on_wait)
                    for k, w in enumerate(waits[:-1]):
                        out.append(
                            mybir.InstNoOp(
                                name=f"{inst.name}-sw{k}",
                                engine=inst.engine,
                                bass_nofuse=True,
                                sync_info=mybir.SyncInfo(
                                    on_wait=[w], on_update=[]
                                ),
                            )
                        )
                    inst.sync_info = mybir.SyncInfo(
                        on_wait=[waits[-1]], on_update=list(si.on_update)
                    )
                    changed = True
                out.append(inst)
            if changed:
                bb.instructions = out


# ------------------------------------------------------------------ cayley
def _cayley_np() -> np.ndarray:
    masks = [0b000, 0b001, 0b010, 0b100, 0b011, 0b101, 0b110, 0b111]
    idx = {m: i for i, m in enumerate(masks)}
    C = np.zeros((8, 8, 8), dtype=np.float32)
    for i, a in enumerate(masks):
        for j, b in enumerate(masks):
            s, aa = 0, a >> 1
            while aa:
                s += bin(aa & b).count("1")
                aa >>= 1
            C[i, j, idx[a ^ b]] = -1.0 if (s % 2) else 1.0
    return C


# ------------------------------------------------------------- phase A build
def build_phase_a(repeat: int = 1) -> bass.Bass:
    """Per-core: x [N, D] f32 -> sums [1, D] f32 (column sums over N)."""
    nc = bass.Bass("TRN2", target_bir_lowering=False, debug=False, num_devices=8)
    x = nc.dram_tensor("x", [N, D], F32, kind="ExternalInput")
    sums = nc.dram_tensor("sums", [1, D], F32, kind="ExternalOutput")

    G = 16                    # groups of 8 row-tiles
    GN = NT // G              # 8 tiles per group
    xg = x[:].rearrange("(g n p) d -> g p n d", g=G, n=GN, p=P)

    with tile.TileContext(nc) as tc:
        with (
            tc.tile_pool(name="grp", bufs=5) as grp,
            tc.tile_pool(name="slots", bufs=1) as slots_pool,
            tc.tile_pool(name="fin", bufs=1) as fin,
            tc.tile_pool(name="ps", bufs=1, space="PSUM") as ps,
        ):
            for rep in range(repeat):
                slots = slots_pool.tile([P, G * D], F32)
                for g in range(G):
                    t = grp.tile([P, GN * D], F32)
                    tv = t[:].rearrange("p (n d) -> p n d", n=GN)
                    nc.sync.dma_start(tv, xg[g])
                    # reduce over n (viewed innermost) -> [P, D]
                    nc.vector.reduce_sum(
                        slots[:, bass.ts(g, D)],
                        t[:].rearrange("p (n d) -> p d n", n=GN),
                        axis=mybir.AxisListType.X,
                    )
                acc = fin.tile([P, D], F32)
                nc.vector.reduce_sum(
                    acc[:],
                    slots[:].rearrange("p (g d) -> p d g", g=G),
                    axis=mybir.AxisListType.X,
                )
                ones = fin.tile([P, 1], F32)
                nc.vector.memset(ones[:], 1.0)
                psum = ps.tile([1, D], F32)
                nc.tensor.matmul(psum[:], ones[:], acc[:], start=True, stop=True)
                out_sb = fin.tile([1, D], F32)
                nc.scalar.copy(out_sb[:], psum[:])
                nc.sync.dma_start(sums[:], out_sb[:])
    return nc


# ----------------------------------------------------------- phase B v2 build
def build_phase_b2(repeat: int = 1, out_bf16: bool = True) -> bass.Bass:
    """Main kernel, v2: y is accumulated entirely in PSUM
    (cc-broadcast + pT.T@Bmat in bf16, + x via an fp32r identity matmul),
    LN stats via bn_stats straight from PSUM, normalize from PSUM with
    the per-partition scale/bias (alternating ACT/DVE for balance)."""
    nc = bass.Bass("TRN2", target_bir_lowering=False, debug=False, num_devices=8)
    ODT = BF16 if out_bf16 else F32
    F32R = mybir.dt.float32r
    x = nc.dram_tensor("x", [N, D], F32R, kind="ExternalInput")
    wpt = nc.dram_tensor("wpt", [P, 4 * MV], BF16, kind="ExternalInput")
    bmat = nc.dram_tensor("bmat", [MV, D], BF16, kind="ExternalInput")
    ccb = nc.dram_tensor("ccb", [1, D], BF16, kind="ExternalInput")
    ident = nc.dram_tensor("ident", [P, P], F32, kind="ExternalInput")
    identr = nc.dram_tensor("identr", [P, P], F32R, kind="ExternalInput")
    y = nc.dram_tensor("y", [N, D], ODT, kind="ExternalOutput")

    S = 2
    NS = NT // S
    x4 = x[:].rearrange("(t s p) d -> t p s d", s=S, p=P)
    y4 = y[:].rearrange("(t s p) d -> t p s d", s=S, p=P)

    with tile.TileContext(nc) as tc:
        with (
            tc.tile_pool(name="const", bufs=1) as cpool,
            tc.tile_pool(name="xin", bufs=3) as xin,
            tc.tile_pool(name="xtp", bufs=2, space="PSUM") as xtp,
            tc.tile_pool(name="xtb", bufs=2) as xtb_pool,
            tc.tile_pool(name="ptp", bufs=2, space="PSUM") as ptp,
            tc.tile_pool(name="ptb", bufs=2) as ptb_pool,
            tc.tile_pool(name="yp", bufs=4, space="PSUM") as yp,
            tc.tile_pool(name="st", bufs=4) as st,
            tc.tile_pool(name="outp", bufs=2) as outp,
        ):
            wpt_sb = cpool.tile([P, 4 * MV], BF16)
            nc.sync.dma_start(wpt_sb[:], wpt[:])
            bmat_sb = cpool.tile([MV, D], BF16)
            nc.sync.dma_start(bmat_sb[:], bmat[:])
            ccb_sb = cpool.tile([1, D], BF16)
            nc.sync.dma_start(ccb_sb[:], ccb[:])
            ident_sb = cpool.tile([P, P], F32)
            nc.sync.dma_start(ident_sb[:], ident[:])
            identr_sb = cpool.tile([P, P], F32R)
            nc.sync.dma_start(identr_sb[:], identr[:])
            ones1 = cpool.tile([1, P], BF16)
            nc.vector.memset(ones1[:], 1.0)
            epsb = cpool.tile([P, 1], F32)
            nc.vector.memset(epsb[:], EPS)

            for rep in range(repeat):
                for i in range(NS):
                    xt = xin.tile([P, S * D], F32R)
                    nc.sync.dma_start(
                        xt[:].rearrange("p (s d) -> p s d", s=S), x4[i]
                    )

                    # xT blocks; xTb free layout = (c, s, 128)
                    xTb = xtb_pool.tile([P, 4 * S * P], BF16)
                    xTbv = xTb[:].rearrange("p (c s q) -> p c s q", c=4, s=S)
                    for s in range(S):
                        xT = xtp.tile([P, D], F32)
                        for c in range(4):
                            nc.tensor.transpose(
                                xT[:, bass.ts(c, P)],
                                xt[:, s * D + c * P : s * D + (c + 1) * P]
                                .bitcast(F32),
                                ident_sb[:],
                            )
                        nc.scalar.copy(
                            xTbv[:, :, s, :],
                            xT[:].rearrange("p (c q) -> p c q", c=4),
                        )

                    # pT[8, S*P] = Wp @ xT
                    pT = ptp.tile([MV, S * P], F32)
                    for c in range(4):
                        nc.tensor.matmul(
                            pT[:],
                            wpt_sb[:, bass.ts(c, MV)],
                            xTb[:, bass.ts(c, S * P)],
                            start=(c == 0),
                            stop=(c == 3),
                        )
                    pTb = ptb_pool.tile([MV, S * P], BF16)
                    nc.scalar.copy(pTb[:], pT[:])

                    bst = st.tile([P, S * 6], F32)
                    ypsums = []
                    for s in range(S):
                        # y_psum = bcast(cc) + pT_s.T @ Bmat + x (fp32r)
                        ypsum = yp.tile([P, D], F32)
                        ypsums.append(ypsum)
                        nc.tensor.matmul(
                            ypsum[:], ones1[:], ccb_sb[:], start=True, stop=False
                        )
                        nc.tensor.matmul(
                            ypsum[:], pTb[:, bass.ts(s, P)], bmat_sb[:],
                            start=False, stop=False,
                        )
                        nc.tensor.matmul(
                            ypsum[:],
                            identr_sb[:],
                            xt[:, bass.ts(s, D)],
                            start=False, stop=True,
                            skip_group_check=True,
                        )
                        nc.vector.bn_stats(bst[:, bass.ts(s, 6)], ypsum[:])

                    mv = st.tile([P, S * 2], F32)
                    for s in range(S):
                        nc.vector.bn_aggr(
                            mv[:, bass.ts(s, 2)], bst[:, bass.ts(s, 6)]
                        )
                    mvv = mv[:].rearrange("p (s two) -> p two s", s=S)
                    std2 = st.tile([P, S], F32)
                    nc.scalar.activation(
                        std2[:], mvv[:, 1, :], mybir.ActivationFunctionType.Sqrt,
                        bias=epsb[:], scale=1.0,
                    )
                    rstd2 = st.tile([P, S], F32)
                    nc.vector.reciprocal(rstd2[:], std2[:])
                    nb2 = st.tile([P, S], F32)
                    nc.vector.tensor_mul(nb2[:], mvv[:, 0, :], rstd2[:])
                    nc.vector.tensor_scalar_mul(nb2[:], nb2[:], -1.0)

                    # normalize straight from PSUM; alternate engines
                    osb = outp.tile([P, S * D], ODT)
                    for s in range(S):
                        if (i + s) % 2 == 0:
                            nc.scalar.activation(
                                osb[:, bass.ts(s, D)], ypsums[s][:],
                                mybir.ActivationFunctionType.Identity,
                                bias=nb2[:, s : s + 1], scale=rstd2[:, s : s + 1],
                            )
                        else:
                            nc.vector.tensor_scalar(
                                osb[:, bass.ts(s, D)], ypsums[s][:],
                                rstd2[:, s : s + 1], nb2[:, s : s + 1],
                                op0=mybir.AluOpType.mult,
                                op1=mybir.AluOpType.add,
                            )
                    nc.sync.dma_start(
                        y4[i], osb[:].rearrange("p (s d) -> p s d", s=S)
                    )
    return nc


# ------------------------------------------------------------- phase B build
def build_phase_b(repeat: int = 1) -> bass.Bass:
    """Per-core main kernel: x [N,D] f32 + per-batch rank-8 params -> out."""
    nc = bass.Bass("TRN2", target_bir_lowering=False, debug=False, num_devices=8)
    x = nc.dram_tensor("x", [N, D], F32, kind="ExternalInput")
    wpt = nc.dram_tensor("wpt", [P, 4 * MV], BF16, kind="ExternalInput")
    bmat = nc.dram_tensor("bmat", [MV, D], BF16, kind="ExternalInput")
    ccb = nc.dram_tensor("ccb", [1, D], BF16, kind="ExternalInput")
    ident = nc.dram_tensor("ident", [P, P], F32, kind="ExternalInput")
    y = nc.dram_tensor("y", [N, D], F32, kind="ExternalOutput")

    S = 2                      # row-tiles per super-tile
    NS = NT // S
    x4 = x[:].rearrange("(t s p) d -> t p s d", s=S, p=P)
    y4 = y[:].rearrange("(t s p) d -> t p s d", s=S, p=P)

    with tile.TileContext(nc) as tc:
        with (
            tc.tile_pool(name="const", bufs=1) as cpool,
            tc.tile_pool(name="xin", bufs=6) as xin,
            tc.tile_pool(name="xtp", bufs=3, space="PSUM") as xtp,
            tc.tile_pool(name="xtb", bufs=4) as xtb_pool,
            tc.tile_pool(name="ptp", bufs=2, space="PSUM") as ptp,
            tc.tile_pool(name="ptb", bufs=4) as ptb_pool,
            tc.tile_pool(name="yp", bufs=3, space="PSUM") as yp,
            tc.tile_pool(name="ysb", bufs=4) as ysb_pool,
            tc.tile_pool(name="st", bufs=8) as st,
            tc.tile_pool(name="outp", bufs=4) as outp,
        ):
            wpt_sb = cpool.tile([P, 4 * MV], BF16)
            nc.sync.dma_start(wpt_sb[:], wpt[:])
            bmat_sb = cpool.tile([MV, D], BF16)
            nc.sync.dma_start(bmat_sb[:], bmat[:])
            ccb_sb = cpool.tile([1, D], BF16)
            nc.sync.dma_start(ccb_sb[:], ccb[:])
            ident_sb = cpool.tile([P, P], F32)
            nc.sync.dma_start(ident_sb[:], ident[:])
            ones1 = cpool.tile([1, P], BF16)
            nc.vector.memset(ones1[:], 1.0)
            epsb = cpool.tile([P, 1], F32)
            nc.vector.memset(epsb[:], EPS)

            for rep in range(repeat):
                for i in range(NS):
                    xt = xin.tile([P, S * D], F32)   # [P, (s, d)]
                    nc.sync.dma_start(
                        xt[:].rearrange("p (s d) -> p s d", s=S), x4[i]
                    )

                    # xT blocks; xTb free layout = (c, s, 128): chunk-major so
                    # the p-matmul rhs for chunk c is one contiguous [P, S*128]
                    xTb = xtb_pool.tile([P, 4 * S * P], BF16)
                    xTbv = xTb[:].rearrange("p (c s q) -> p c s q", c=4, s=S)
                    for s in range(S):
                        xT = xtp.tile([P, D], F32)
                        for c in range(4):
                            nc.tensor.transpose(
                                xT[:, bass.ts(c, P)],
                                xt[:, s * D + c * P : s * D + (c + 1) * P],
                                ident_sb[:],
                            )
                        # strided copy: chunk c of this s -> xTb[:, c, s, :]
                        nc.scalar.copy(
                            xTbv[:, :, s, :],
                            xT[:].rearrange("p (c q) -> p c q", c=4),
                        )

                    # pT[8, S*P] = Wp @ xT  (accumulate over 4 d-chunks)
                    pT = ptp.tile([MV, S * P], F32)
                    for c in range(4):
                        nc.tensor.matmul(
                            pT[:],
                            wpt_sb[:, bass.ts(c, MV)],
                            xTb[:, bass.ts(c, S * P)],
                            start=(c == 0),
                            stop=(c == 3),
                        )
                    pTb = ptb_pool.tile([MV, S * P], BF16)
                    nc.scalar.copy(pTb[:], pT[:])

                    ysb = ysb_pool.tile([P, S * D], F32)
                    bst = st.tile([P, S * 6], F32)
                    for s in range(S):
                        # y_psum = broadcast(cc) + pT_s.T @ Bmat
                        ypsum = yp.tile([P, D], F32)
                        nc.tensor.matmul(
                            ypsum[:], ones1[:], ccb_sb[:], start=True, stop=False
                        )
                        nc.tensor.matmul(
                            ypsum[:], pTb[:, bass.ts(s, P)], bmat_sb[:],
                            start=False, stop=True,
                        )
                        # y = x + y_psum
                        nc.vector.scalar_tensor_tensor(
                            ysb[:, bass.ts(s, D)], xt[:, bass.ts(s, D)], 1.0,
                            ypsum[:],
                            op0=mybir.AluOpType.mult, op1=mybir.AluOpType.add,
                        )
                        # LN stats in one pass
                        nc.vector.bn_stats(
                            bst[:, bass.ts(s, 6)], ysb[:, bass.ts(s, D)]
                        )

                    # aggregate -> [mean, var] per sub-tile; batched tiny ops
                    mv = st.tile([P, S * 2], F32)
                    for s in range(S):
                        nc.vector.bn_aggr(
                            mv[:, bass.ts(s, 2)], bst[:, bass.ts(s, 6)]
                        )
                    mvv = mv[:].rearrange("p (s two) -> p two s", s=S)
                    means = mvv[:, 0, :]   # [P, S] strided
                    vars_ = mvv[:, 1, :]   # [P, S] strided
                    std2 = st.tile([P, S], F32)
                    nc.scalar.activation(
                        std2[:], vars_, mybir.ActivationFunctionType.Sqrt,
                        bias=epsb[:], scale=1.0,
                    )
                    rstd2 = st.tile([P, S], F32)
                    nc.vector.reciprocal(rstd2[:], std2[:])
                    nb2 = st.tile([P, S], F32)
                    nc.vector.tensor_mul(nb2[:], means, rstd2[:])
                    nc.vector.tensor_scalar_mul(nb2[:], nb2[:], -1.0)

                    # normalize on DVE: out = y*rstd - mu*rstd
                    osb = outp.tile([P, S * D], F32)
                    for s in range(S):
                        nc.vector.tensor_scalar(
                            osb[:, bass.ts(s, D)], ysb[:, bass.ts(s, D)],
                            rstd2[:, s : s + 1], nb2[:, s : s + 1],
                            op0=mybir.AluOpType.mult, op1=mybir.AluOpType.add,
                        )
                    nc.sync.dma_start(
                        y4[i], osb[:].rearrange("p (s d) -> p s d", s=S)
                    )
    return nc


# ------------------------------------------------------------------- driver
_CACHE: dict = {}


def _get_nc(which: str, repeat: int = 1) -> bass.Bass:
    key = (which, repeat)
    if key not in _CACHE:
        if which == "a":
            nc = build_phase_a(repeat)
        elif which == "b":
            nc = build_phase_b(repeat)
        elif which == "b2":
            nc = build_phase_b2(repeat, out_bf16=True)
        elif which == "b2f":
            nc = build_phase_b2(repeat, out_bf16=False)
        else:
            raise KeyError(which)
        _split_multi_waits(nc)
        _CACHE[key] = nc
    return _CACHE[key]


def host_algebra(sums: np.ndarray, Wp, bp, Wm, bm, Wo, bo):
    """sums: [B, D] column sums -> per-batch bf16 operands for phase B."""
    C = _cayley_np()
    mean = sums / float(N)                              # [B, D]
    mf = mean @ Wm.T + bm                               # [B, 8]
    M = np.einsum("ijk,bj->bik", C, mf)                 # [B, 8, 8]
    Bmat = SCALE * (M @ Wo.T)                           # [B, 8, D]
    cc = np.einsum("k,bkd->bd", bp, Bmat) + SCALE * bo  # [B, D]
    wpt = np.ascontiguousarray(
        Wp.T.reshape(4, P, MV).transpose(1, 0, 2).reshape(P, 4 * MV)
    )                                                   # [P, 4*MV]; wpt[p,c,m]=Wp[m,c*128+p]
    return (
        wpt.astype(BF16_NP),
        Bmat.astype(BF16_NP),
        cc.astype(BF16_NP),
    )


MAIN_KERNEL = "b"   # which phase-B variant kernel() uses


def _run_with_retry(nc, in_maps, tries: int = 3):
    """The axon-tunneled device occasionally wedges
    (NRT_EXEC_UNIT_UNRECOVERABLE) and auto-recovers; retry."""
    import time as _time

    for attempt in range(tries):
        try:
            return run_bass_kernel_spmd(
                nc, in_maps, core_ids=_CORES, trace=False
            )
        except Exception:
            if attempt == tries - 1:
                raise
            _time.sleep(20.0)


def kernel(x, Wp, bp, Wm, bm, Wo, bo, gamma, beta):
    x = np.ascontiguousarray(np.asarray(x, dtype=np.float32))
    Wp, bp, Wm, bm, Wo, bo, gamma, beta = (
        np.asarray(a, dtype=np.float32) for a in (Wp, bp, Wm, bm, Wo, bo, gamma, beta)
    )

    nc_a = _get_nc("a")
    res_a = _run_with_retry(
        nc_a, [{"x": x[b]} for b in range(B)]
    )
    sums = np.stack([res_a.results[b]["sums"][0] for b in range(B)])  # [B, D]

    wpt, Bmat, cc = host_algebra(sums, Wp, bp, Wm, bm, Wo, bo)
    ident = np.eye(P, dtype=np.float32)

    nc_b = _get_nc(MAIN_KERNEL)
    in_maps = [
        {
            "x": x[b],
            "wpt": wpt,
            "bmat": np.ascontiguousarray(Bmat[b]),
            "ccb": cc[b][None, :],
            "ident": ident,
            "identr": ident,
        }
        for b in range(B)
    ]
    if MAIN_KERNEL == "b":
        for m in in_maps:
            del m["identr"]
    res_b = _run_with_retry(nc_b, in_maps)
    out = np.stack(
        [np.asarray(res_b.results[b]["y"], dtype=np.float32) for b in range(B)]
    )  # [B, N, D]

    if not (np.all(gamma == 1.0) and np.all(beta == 0.0)):
        out = out * gamma + beta
    return np.ascontiguousarray(out, dtype=np.float32)
